# revision 18
# baseline (speedup 1.0000x reference)
"""Trainium2 Bass kernel for two-stream cross-attention (v5).

Reference computation (per batch b):
    qkv_s = x_s @ W_qkv_s ; split into q_s, k_s, v_s (16 heads x 64)
    dir1: out1 = softmax(q2 k1^T * scale) v1, merged @ W_out1 + b_out1
    dir2: out2 = softmax(q1 k2^T * scale) v2, merged @ W_out2 + b_out2

Sharding: 8 cores = 2 batches x 4 head-groups (4 heads each). Each core
computes q/k/v for its 4 heads (both streams), both attention directions,
and a partial output projection (row-block of W_out). Host transposes x
(so the device only does linear DMA) and sums the 4 f16 partials per
batch, adding the bias.

Kernel structure (all matmuls bf16, fp32 PSUM accumulation):
  - Heads processed in row-tiled PAIRS: head 2*cb on partitions 0-63,
    head 2*cb+1 on 64-127. The pair's two S^T matmuls (K=64) carry
    tile_position (0,0)/(64,0), run CONCURRENTLY in the PE array into
    different PSUM banks -> 2x S throughput.
  - Flash-style inner loop per key block: S-pair -> one exp[128,1024]
    covering both heads -> 8 AV matmuls accumulating into per-head
    oav[128,4,72] PSUM banks (appended ones-column = softmax rowsum;
    only the first matmul per bank carries start=True since start
    clears has_written bank-wide).
  - ScalarE (exp) is the critical engine (~270us). All other qkv work
    (v of both streams, kT1, qT0) is emitted in dataflow order but at
    hugely deprioritized scheduler priority, so the PE executes it only
    when the Scalar-bound attention pipeline has no tensor work ready.
  - Dummy warmup matmuls (emitted first, no DMA deps) keep the PE HAM
    clock at 2.4 GHz through the initial DMA window.
  - PSUM pools are partitioned by instruction stream (slot grants are
    FIFO in emission order): st 4 banks, oav 2, ptr/outproj 1, fills 1.
"""

import os

import numpy as np
import ml_dtypes

import concourse.bass as bass
import concourse.mybir as mybir
import concourse.tile as tile
from concourse import bacc
from concourse.bass_utils import run_bass_kernel_spmd
from concourse.masks import make_identity

BF16 = mybir.dt.bfloat16
F16 = mybir.dt.float16
F32 = mybir.dt.float32

B, N, DIM = 2, 2048, 1024
HEADS, DH = 16, 64
HPC = 4                      # heads per core
HC = HPC * DH                # 256 inner columns per core
SCALE = DH ** -0.5
P = 128
FB = DIM // P                # 8 feature blocks
KB = N // P                  # 16 key blocks
QT = 512                     # q-tile
NQT = N // QT                # 4 q-tiles
NM = QT // P                 # 4 m-blocks per q-tile

NCORES = 8
LOWPRI = 10_000_000          # negative high_priority offset for fill work

_NC = None
LAST_RESULTS = None


def _build():
    nc = bacc.Bacc(None, target_bir_lowering=False, debug=False, num_devices=NCORES)

    # x is pre-transposed on the host: x^T [DIM, N] -> linear DMA loads
    xs = [nc.dram_tensor(f"x{s + 1}", [DIM, N], BF16, kind="ExternalInput")
          for s in range(2)]
    ws = [nc.dram_tensor(f"w{s + 1}", [DIM, 3 * HC], BF16, kind="ExternalInput")
          for s in range(2)]
    wos = [nc.dram_tensor(f"wo{s + 1}", [HC, DIM], BF16, kind="ExternalInput")
           for s in range(2)]
    os_ = [nc.dram_tensor(f"o{d + 1}", [N, DIM], F16, kind="ExternalOutput")
           for d in range(2)]

    with tile.TileContext(nc) as tc:
        with (
            tc.tile_pool(name="const", bufs=1) as const_pool,
            tc.tile_pool(name="qkv", bufs=1) as qkv_pool,
        ):
            identity = const_pool.tile([P, P], BF16)
            make_identity(nc, identity[:])
            wo_sb = [const_pool.tile([P, 2, DIM], BF16, name=f"wo{d}")
                     for d in range(2)]
            for d in range(2):
                for cb in range(2):
                    nc.sync.dma_start(
                        wo_sb[d][:, cb, :], wos[d][cb * P:(cb + 1) * P, :])

            # persistent per-stream q/k/v (bf16) and per-dir O^T
            qT = [qkv_pool.tile([P, 2, N], BF16, name=f"qT{s}") for s in range(2)]
            kT = [qkv_pool.tile([P, 2, N], BF16, name=f"kT{s}") for s in range(2)]
            vx = [qkv_pool.tile([P, KB, HPC, DH + 1], BF16, name=f"vx{s}")
                  for s in range(2)]
            ot = [qkv_pool.tile([P, 2, N], BF16, name=f"ot{d}") for d in range(2)]
            for s in range(2):
                nc.vector.memset(vx[s][:, :, :, DH], 1.0)

            with (
                tc.tile_pool(name="xT", bufs=1) as xt_pool,
                tc.tile_pool(name="wsb", bufs=1) as w_pool,
                tc.tile_pool(name="pmm", bufs=1, space="PSUM") as pmm_pool,
                tc.tile_pool(name="st", bufs=2, space="PSUM") as st_pool,
                tc.tile_pool(name="oav", bufs=2, space="PSUM") as oav_pool,
                tc.tile_pool(name="ptrpop", bufs=1, space="PSUM") as ptr_pool,
                tc.tile_pool(name="pt", bufs=4) as pt_pool,
                tc.tile_pool(name="osb", bufs=4) as osb_pool,
                tc.tile_pool(name="rec", bufs=4) as rec_pool,
                tc.tile_pool(name="ost", bufs=3) as ost_pool,
            ):
                # HAM warmup: no-dep dummy matmuls, first in the PE queue.
                # They run back-to-back during the initial DMA wait, putting
                # the PE clock at 2.4 GHz before the real work arrives.
                warm = pmm_pool.tile([P, P], F32, name="warm", tag="pmm")
                for _ in range(120):
                    nc.tensor.matmul(warm[:], identity[:], identity[:],
                                     start=True, stop=True)

                xT = [xt_pool.tile([P, FB, N], BF16, name=f"xT{s}")
                      for s in range(2)]
                w_sb = [w_pool.tile([P, FB, 3 * HC], BF16, name=f"w{s}")
                        for s in range(2)]
                # interleave both streams' loads across the DMA queues;
                # stream 1 slightly first (dir0 needs qT[1] + kT[0])
                for fb in range(FB):
                    for s in (1, 0):
                        nc.sync.dma_start(
                            w_sb[s][:, fb, :], ws[s][fb * P:(fb + 1) * P, :])
                        nc.sync.dma_start(
                            xT[s][:, fb, :], xs[s][fb * P:(fb + 1) * P, :])

                def qk_group(s, off, cb, nt, dest, pool, tag):
                    ps = pool.tile([P, 512], F32, name="pqk", tag=tag)
                    for fb in range(FB):
                        nc.tensor.matmul(
                            ps[:],
                            w_sb[s][:, fb, off + cb * P:off + (cb + 1) * P],
                            xT[s][:, fb, nt * 512:(nt + 1) * 512],
                            start=(fb == 0), stop=(fb == FB - 1))
                    nc.vector.tensor_copy(dest[:, cb, nt * 512:(nt + 1) * 512],
                                          ps[:])

                def v_group(s, kb, pool, tag):
                    ps = pool.tile([P, HC], F32, name="pv", tag=tag)
                    for fb in range(FB):
                        nc.tensor.matmul(
                            ps[:],
                            xT[s][:, fb, kb * P:(kb + 1) * P],
                            w_sb[s][:, fb, 2 * HC:3 * HC],
                            start=(fb == 0), stop=(fb == FB - 1))
                    nc.vector.tensor_copy(
                        vx[s][:, kb, :, 0:DH],
                        ps[:].rearrange("p (h d) -> p h d", h=HPC))

                # ---- head: the minimum qkv for dir0 to start ----
                # st pool stays attention-only (slot grants are FIFO: any
                # head tile there would delay the first S-pair). qT1+v0 go
                # through the pmm slot nt-major (first unit's needs first);
                # kT0 pipelines through the idle oav slots.
                for nt in range(4):
                    for cb in range(2):
                        qk_group(1, 0, cb, nt, qT[1], pmm_pool, "pmm")
                    for kb in range(4 * nt, 4 * nt + 4):
                        v_group(0, kb, pmm_pool, "pmm")
                for nt in range(4):
                    for cb in range(2):
                        qk_group(0, HC, cb, nt, kT[0], oav_pool, "oav")

                # ---- remaining qkv, emitted in dataflow order BUT at very
                # low scheduler preference: executes only in PE gaps of the
                # Scalar-bound attention stream. Order staged so dir1's
                # earliest needs (kT1/qT0 nt0, first v1 blocks) come first.
                with tc.high_priority(offset=-LOWPRI):
                    for cb in range(2):
                        qk_group(0, 0, cb, 0, qT[0], pmm_pool, "pmm")
                        qk_group(1, HC, cb, 0, kT[1], pmm_pool, "pmm")
                    for nt in range(4):
                        for kb in range(4 * nt, 4 * nt + 4):
                            v_group(1, kb, pmm_pool, "pmm")
                        if nt > 0:
                            for cb in range(2):
                                qk_group(1, HC, cb, nt, kT[1], pmm_pool, "pmm")
                    for nt in range(1, 4):
                        for cb in range(2):
                            qk_group(0, 0, cb, nt, qT[0], pmm_pool, "pmm")

                def attn_unit(d, qs, ks, qt, cb):
                    """Head pair (2*cb, 2*cb+1), queries qt*QT..+QT."""
                    q_t, k_t, v_t = qT[qs], kT[ks], vx[ks]
                    q0 = qt * QT
                    oav = [oav_pool.tile([P, NM, 72], F32, name="oav",
                                         tag="oav")
                           for _ in range(2)]
                    for kb in range(KB):
                        st = st_pool.tile([P, 2, QT], F32, name="st", tag="st")
                        for hh in range(2):
                            po = hh * DH
                            nc.tensor.matmul(
                                st[:, hh, :],
                                k_t[po:po + DH, cb, kb * P:(kb + 1) * P],
                                q_t[po:po + DH, cb, q0:q0 + QT],
                                start=True, stop=True)
                        pt = pt_pool.tile([P, 2, QT], BF16, name="pt")
                        nc.scalar.activation(
                            pt[:], st[:],
                            mybir.ActivationFunctionType.Exp, scale=SCALE)
                        for hh in range(2):
                            head = 2 * cb + hh
                            for m in range(NM):
                                nc.tensor.matmul(
                                    oav[hh][:, m, 0:DH + 1],
                                    pt[:, hh, m * P:(m + 1) * P],
                                    v_t[:, kb, head, :],
                                    start=(kb == 0 and m == 0),
                                    stop=(kb == KB - 1 and m == NM - 1),
                                    skip_group_check=True)
                    # normalize, transpose O -> O^T, write into ot
                    ptr = ptr_pool.tile([DH, 2 * NM, P], BF16, name="ptr",
                                        tag="ptrpop")
                    for hh in range(2):
                        for m in range(NM):
                            rec = rec_pool.tile([P, 1], F32, name="rec")
                            nc.vector.reciprocal(rec[:], oav[hh][:, m, DH:DH + 1])
                            osb = osb_pool.tile([P, DH], BF16, name="osb")
                            nc.vector.tensor_scalar_mul(
                                osb[:], oav[hh][:, m, 0:DH], rec[:])
                            nc.tensor.transpose(
                                ptr[:, hh * NM + m, :], osb[:], identity[:])
                    for hh in range(2):
                        po = hh * DH
                        nc.vector.tensor_copy(
                            ot[d][po:po + DH, cb, q0:q0 + QT],
                            ptr[:, hh * NM:(hh + 1) * NM, :])

                def outproj(d, qt):
                    for mb in range(NM):
                        row = qt * QT + mb * P
                        ost = ost_pool.tile([P, DIM], F16, name="ost")
                        for nb in range(2):
                            pop = ptr_pool.tile([P, 512], F32, name="pop",
                                                tag="ptrpop")
                            for cb2 in range(2):
                                nc.tensor.matmul(
                                    pop[:],
                                    ot[d][:, cb2, row:row + P],
                                    wo_sb[d][:, cb2, nb * 512:(nb + 1) * 512],
                                    start=(cb2 == 0), stop=(cb2 == 1))
                            nc.vector.tensor_copy(ost[:, nb * 512:(nb + 1) * 512],
                                                  pop[:])
                        nc.sync.dma_start(os_[d][row:row + P, :], ost[:])

                # ---- attention + inline output projection ----
                for qt in range(NQT):
                    for cb in range(2):
                        attn_unit(0, 1, 0, qt, cb)
                    outproj(0, qt)
                for qt in range(NQT):
                    for cb in range(2):
                        attn_unit(1, 0, 1, qt, cb)
                    outproj(1, qt)

    nc.compile()
    return nc


def _shard_inputs(x1, x2, W_qkv1, W_qkv2, W_out1, W_out2):
    bf = ml_dtypes.bfloat16
    in_maps = []
    xs = [np.ascontiguousarray(x1).astype(bf), np.ascontiguousarray(x2).astype(bf)]
    w_full = [np.asarray(W_qkv1), np.asarray(W_qkv2)]
    wo_full = [np.asarray(W_out1), np.asarray(W_out2)]
    for cid in range(NCORES):
        b, g = divmod(cid, 4)
        cs = slice(g * HC, (g + 1) * HC)
        m = {}
        for s in range(2):
            m[f"x{s + 1}"] = np.ascontiguousarray(xs[s][b].T)
            w = w_full[s]
            m[f"w{s + 1}"] = np.ascontiguousarray(np.concatenate(
                [w[:, 0:DIM][:, cs], w[:, DIM:2 * DIM][:, cs],
                 w[:, 2 * DIM:3 * DIM][:, cs]], axis=1)).astype(bf)
            m[f"wo{s + 1}"] = np.ascontiguousarray(wo_full[s][cs, :]).astype(bf)
        in_maps.append(m)
    return in_maps


def kernel(x1, x2, W_qkv1, W_qkv2, W_out1, b_out1, W_out2, b_out2):
    global _NC, LAST_RESULTS
    if _NC is None:
        _NC = _build()

    in_maps = _shard_inputs(x1, x2, W_qkv1, W_qkv2, W_out1, W_out2)
    trace = bool(os.environ.get("BASS_KERNEL_TRACE"))
    res = run_bass_kernel_spmd(_NC, in_maps, list(range(NCORES)), trace=trace)
    LAST_RESULTS = res

    out1 = np.zeros((B, N, DIM), np.float32)
    out2 = np.zeros((B, N, DIM), np.float32)
    for cid in range(NCORES):
        b = cid // 4
        out1[b] += res.results[cid]["o1"].astype(np.float32)
        out2[b] += res.results[cid]["o2"].astype(np.float32)
    out1 += np.asarray(b_out1, np.float32)
    out2 += np.asarray(b_out2, np.float32)
    return out1, out2
